# revision 1
# baseline (speedup 1.0000x reference)
"""Trainium2 Bass kernel: 2-layer LSTM decoder with embedding lookup.

Reference computation (per nn.Decoder):
    tgt_embed = emb[prev_tgt_tokens]                      # [B, T, D]
    for t in 0..T-1:
        x = tgt_embed[:, t]
        for l in 0..1:
            gates = x @ W_ih[l].T + b_ih[l] + h[l] @ W_hh[l].T + b_hh[l]
            i, f, g, o = split(gates, 4)
            c[l] = sigmoid(f) * c[l] + sigmoid(i) * tanh(g)
            h[l] = sigmoid(o) * tanh(c[l])
            x = h[l]
        out[:, t] = h[1]

Sharding: data-parallel over batch B=64 across 8 cores (8 rows each);
weights replicated; the sequential time loop runs on-device per core.

Kernel design (per core), v2:
  - fp16 operand/activation datatypes throughout (PSUM accumulates fp32);
    rel-err budget is 2e-2, fp16 keeps it ~1e-3.
  - Phase A: embedding gather (indirect DMA) + batched input projection
    x @ W_ih0.T over all T steps at full M=128 PE utilization, spilled to
    a DRAM scratch as fp16.
  - Layer-1 input projections h0 @ W_ih1.T are batched over 16-step
    windows (also M=128), not streamed per step.  Layer 1 lags layer 0 by
    LAG steps; one stacked activation chain serves both layers.
  - Recurrent matmuls use 128x32 column tiling: L0 accumulates into PSUM
    rows 0:8 (array col group 0) while L1 accumulates into rows 32:40
    (group 32) concurrently.  Gate banks f|i and g|o sit side by side in
    two [128,1024] PSUM tiles.
  - The x-projection gx enters PSUM via a tiny identity matmul (K=16)
    instead of DVE adds; the g-gate weights/biases are pre-doubled
    host-side so a single plain sigmoid per tile computes sigma(2 z_g)
    without a per-row scale or an ACT table switch.
  - h -> h^T transposes run on the DMA XBAR (16-bit transpose), off the
    PE/DVE critical engines.
"""

import os

import numpy as np

import concourse.bass as bass
import concourse.mybir as mybir
import concourse.tile as tile
from concourse import bacc
from concourse.bass_utils import run_bass_kernel_spmd
from concourse.masks import make_identity

N_CORES = 8
B = 64
T = int(os.environ.get("BASS_LSTM_T", "128"))
D = 512
V = 32000
G = 2048            # 4*D gate dims per layer
BL = B // N_CORES   # 8 batch rows per core
KC = D // 128       # 4 contraction chunks of 128
MT = BL * T // 128  # M-tiles (128 token rows) for the input matmul
REPS = int(os.environ.get("BASS_LSTM_REPS", "1"))  # timing-only: repeat phase B
WIN = 16            # wih1 batching window (steps)
LAG = 18            # layer-1 step lag behind layer 0
R1 = 32             # partition row where the layer-1 lane starts
NR = R1 + BL        # 40
SR = 48             # hst rows padded for the DMA XBAR transpose (16-mult)
F32 = mybir.dt.float32
F16 = mybir.dt.float16
I32 = mybir.dt.int32
AFT = mybir.ActivationFunctionType

FSL = slice(0, 512)        # f (in pFI) / g (in pGO) columns
ISL = slice(512, 1024)     # i (in pFI) / o (in pGO) columns


def _build():
    nc = bacc.Bacc(
        "TRN2",
        target_bir_lowering=False,
        debug=False,
        enable_asserts=False,
        num_devices=N_CORES,
    )

    tok_d = nc.dram_tensor("tokens", [BL * T, 1], I32, kind="ExternalInput")
    emb_d = nc.dram_tensor("emb", [V, D], F32, kind="ExternalInput")
    wih0_d = nc.dram_tensor("wih0t", [D, G], F16, kind="ExternalInput")
    whh0_d = nc.dram_tensor("whh0t", [D, G], F16, kind="ExternalInput")
    wih1_d = nc.dram_tensor("wih1t", [D, G], F16, kind="ExternalInput")
    whh1_d = nc.dram_tensor("whh1t", [D, G], F16, kind="ExternalInput")
    bias0_d = nc.dram_tensor("bias0", [128, G], F16, kind="ExternalInput")
    bias1_d = nc.dram_tensor("bias1", [1, G], F16, kind="ExternalInput")
    il_d = nc.dram_tensor("il", [16, 64], F16, kind="ExternalInput")
    ht_d = nc.dram_tensor("ht_init", [128, KC * 48], F16, kind="ExternalInput")
    c_d = nc.dram_tensor("c_init", [NR, D], F16, kind="ExternalInput")
    out_d = nc.dram_tensor("out", [BL, T, D], F16, kind="ExternalOutput")

    with tile.TileContext(nc) as tc:
        _body(
            tc,
            tok=tok_d.ap(),
            emb=emb_d.ap(),
            w=[wih0_d.ap(), whh0_d.ap(), wih1_d.ap(), whh1_d.ap()],
            bias0=bias0_d.ap(),
            bias1=bias1_d.ap(),
            il=il_d.ap(),
            ht0=ht_d.ap(),
            c0=c_d.ap(),
            out=out_d.ap(),
        )
    nc.compile()
    return nc


def _body(tc, tok, emb, w, bias0, bias1, il, ht0, c0, out):
    nc = tc.nc
    with (
        tc.tile_pool(name="wpool", bufs=1) as wp,
        tc.tile_pool(name="dram", bufs=1, space="DRAM") as dr,
        tc.tile_pool(name="state", bufs=1) as st,
        tc.tile_pool(name="work", bufs=2) as wk,
        tc.tile_pool(name="pspool", bufs=1, space="PSUM") as pp,
    ):
        # ---- persistent tiles -------------------------------------------
        id_sb = wp.tile([128, 128], F32)
        make_identity(nc, id_sb[:])

        whh0_sb = wp.tile([128, KC * G], F16)
        wih1_sb = wp.tile([128, KC * G], F16)
        whh1_sb = wp.tile([128, KC * G], F16)
        for dst, src in ((whh0_sb, w[1]), (wih1_sb, w[2]), (whh1_sb, w[3])):
            for c in range(KC):
                nc.sync.dma_start(
                    out=dst[:, c * G : (c + 1) * G],
                    in_=src.rearrange("(c p) n -> p c n", p=128)[:, c, :],
                )

        bias1_sb = wp.tile([1, G], F16)
        nc.sync.dma_start(out=bias1_sb[:], in_=bias1)
        il_sb = wp.tile([16, 64], F16)
        nc.sync.dma_start(out=il_sb[:], in_=il)
        ones_sb = wp.tile([1, 128], F16)
        nc.vector.memset(ones_sb[:], 1.0)

        gx_dram = dr.tile([BL * T, G], F16)

        # LSTM state + chain tiles: L0 lane on rows 0:8, L1 on rows 32:40.
        cst = st.tile([NR, D], F16)
        fct = st.tile([NR, D], F16)
        mt_ = st.tile([NR, D], F16)
        tch = st.tile([NR, D], F16)
        gtFI = st.tile([NR, 2 * D], F16)
        gtGO = st.tile([NR, 2 * D], F16)
        hst = st.tile([SR, D], F16)
        for tile_ in (cst, fct, mt_, tch, gtFI, gtGO, hst):
            nc.vector.memset(tile_[:], 0.0)
        nc.sync.dma_start(out=cst[:BL, :], in_=c0[:BL, :])

        xb = [st.tile([128, KC * 48], F16, name=f"xb{i}") for i in range(2)]
        nc.vector.memset(xb[1][:], 0.0)
        nc.sync.dma_start(out=xb[0][:], in_=ht0)
        winT = [st.tile([128, KC * 128], F16, name=f"winT{i}") for i in range(2)]
        gx1buf = [st.tile([128, G], F16, name=f"gx1buf{i}") for i in range(2)]

        # ---- phase A: gather + transpose + batched input projection ----
        with tc.tile_pool(name="ph0", bufs=1) as p0:
            wih0_sb = p0.tile([128, KC * G], F16)
            for c in range(KC):
                nc.sync.dma_start(
                    out=wih0_sb[:, c * G : (c + 1) * G],
                    in_=w[0].rearrange("(c p) n -> p c n", p=128)[:, c, :],
                )
            bias0_bc = p0.tile([128, G], F16)
            nc.sync.dma_start(out=bias0_bc[:], in_=bias0)

            for m in range(MT):
                idx_m = p0.tile([128, 1], I32, tag="idx", bufs=2)
                nc.sync.dma_start(out=idx_m[:], in_=tok[m * 128 : (m + 1) * 128, :])
                emb_m = p0.tile([128, D], F32, tag="embrows", bufs=2)
                nc.gpsimd.indirect_dma_start(
                    out=emb_m[:],
                    out_offset=None,
                    in_=emb,
                    in_offset=bass.IndirectOffsetOnAxis(ap=idx_m[:, :1], axis=0),
                )
                pst = pp.tile([128, 512], F32, tag="ps", bufs=2, name="pst")
                for c in range(KC):
                    nc.tensor.transpose(
                        out=pst[:, c * 128 : (c + 1) * 128],
                        in_=emb_m[:, c * 128 : (c + 1) * 128],
                        identity=id_sb[:],
                    )
                embT_m = p0.tile([128, D], F16, tag="embT", bufs=2)
                nc.vector.tensor_copy(out=embT_m[:], in_=pst[:, 0:512])
                gxs = p0.tile([128, G], F16, tag="gxs", bufs=2)
                for n in range(4):
                    psm = pp.tile([128, 512], F32, tag="ps", bufs=2, name="psm")
                    for c in range(KC):
                        nc.tensor.matmul(
                            out=psm[:, 0:512],
                            lhsT=embT_m[:, c * 128 : (c + 1) * 128],
                            rhs=wih0_sb[:, c * G + n * 512 : c * G + (n + 1) * 512],
                            start=(c == 0),
                            stop=(c == KC - 1),
                        )
                    nc.vector.tensor_add(
                        out=gxs[:, n * 512 : (n + 1) * 512],
                        in0=psm[:, 0:512],
                        in1=bias0_bc[:, n * 512 : (n + 1) * 512],
                    )
                nc.sync.dma_start(
                    out=gx_dram[m * 128 : (m + 1) * 128, :], in_=gxs[:]
                )

        # ---- phase B: recurrent loop ------------------------------------
        # Pre-zero every rotation buffer of the gx stacks: rows 8:16 are
        # read by the inject matmul before layer 1 becomes active, and
        # stale SBUF bits can be fp16 NaNs that poison the c/h state.
        for bi in range(3):
            z1 = wk.tile([16, 1024], F16, tag="gsFI", bufs=3, name="gsFI")
            z2 = wk.tile([16, 1024], F16, tag="gsGO", bufs=3, name="gsGO")
            nc.vector.memset(z1[:], 0.0)
            nc.vector.memset(z2[:], 0.0)
        for _rep in range(REPS):
          for k in range(T + LAG):
              l0 = k < T
              l1 = k >= LAG
              t = k
              j = k - LAG

              # gx stacks: rows 0:8 = L0 step t, rows 8:16 = L1 step j.
              gsFI = wk.tile([16, 1024], F16, tag="gsFI", bufs=3, name="gsFI")
              gsGO = wk.tile([16, 1024], F16, tag="gsGO", bufs=3, name="gsGO")
              if l0:
                  nc.sync.dma_start(
                      out=gsFI[0:8, :], in_=gx_dram[t * 8 : (t + 1) * 8, 0:1024]
                  )
                  nc.sync.dma_start(
                      out=gsGO[0:8, :], in_=gx_dram[t * 8 : (t + 1) * 8, 1024:2048]
                  )
              if l1:
                  wbuf = gx1buf[(j // WIN) % 2]
                  r = (j % WIN) * 8
                  nc.sync.dma_start(out=gsFI[8:16, :], in_=wbuf[r : r + 8, 0:1024])
                  nc.sync.dma_start(out=gsGO[8:16, :], in_=wbuf[r : r + 8, 1024:2048])

              xb_prev = xb[k % 2]
              xb_new = xb[(k + 1) % 2]

              pFI = pp.tile([128, 1024], F32, tag="pFI", name="pFI")
              pGO = pp.tile([128, 1024], F32, tag="pGO", name="pGO")
              # inject gx (+biases) into PSUM rows {0:8, 32:40}; start=True
              # clears each 512-col bank before the whh accumulation.
              for p, gs in ((pFI, gsFI), (pGO, gsGO)):
                  for cs in (FSL, ISL):
                      nc.tensor.matmul(
                          out=p[0:NR, cs],
                          lhsT=il_sb[0:16, 0:NR],
                          rhs=gs[0:16, cs],
                          start=True,
                          stop=False,
                          skip_group_check=True,
                      )
              # recurrent matmuls, 2-wide column-tiled (groups 0 and 32)
              lanes = []
              if l0:
                  lanes.append((whh0_sb, 0))
              if l1:
                  lanes.append((whh1_sb, R1))
              for p, goff in ((pFI, 0), (pGO, 1024)):
                  for gi, cs in ((0, FSL), (1, ISL)):
                      col = goff + gi * 512
                      for li, (wsb, r0) in enumerate(lanes):
                          for c in range(KC):
                              nc.tensor.matmul(
                                  out=p[r0 : r0 + 8, cs],
                                  lhsT=xb_prev[:, c * 48 + r0 : c * 48 + r0 + 8],
                                  rhs=wsb[:, c * G + col : c * G + col + 512],
                                  start=False,
                                  stop=(
                                      c == KC - 1
                                      and gi == 1
                                      and li == len(lanes) - 1
                                  ),
                                  skip_group_check=True,
                              )

              # stacked activation chain (both lanes, rows 0:40)
              nc.scalar.activation(out=gtFI[:, :], in_=pFI[0:NR, :], func=AFT.Sigmoid)
              nc.scalar.activation(out=gtGO[:, :], in_=pGO[0:NR, :], func=AFT.Sigmoid)
              nc.vector.tensor_mul(out=fct[:], in0=gtFI[:, FSL], in1=cst[:])
              nc.vector.tensor_sub(out=fct[:], in0=fct[:], in1=gtFI[:, ISL])
              nc.vector.scalar_tensor_tensor(
                  out=mt_[:], in0=gtGO[:, FSL], scalar=2.0, in1=gtFI[:, ISL],
                  op0=mybir.AluOpType.mult, op1=mybir.AluOpType.mult,
              )
              nc.vector.tensor_add(out=cst[:], in0=fct[:], in1=mt_[:])
              nc.scalar.activation(out=tch[:], in_=cst[:], func=AFT.Sigmoid, scale=2.0)
              nc.vector.tensor_scalar(
                  out=tch[:], in0=tch[:], scalar1=2.0, scalar2=-1.0,
                  op0=mybir.AluOpType.mult, op1=mybir.AluOpType.add,
              )
              nc.vector.tensor_mul(out=hst[0:NR, :], in0=gtGO[:, ISL], in1=tch[:])
              if l1:
                  nc.sync.dma_start(out=out[:, j, :], in_=hst[R1:NR, :])

              # h -> h^T via DMA XBAR transpose (fp16), both lanes at once
              for c in range(KC):
                  nc.sync.dma_start(
                      out=xb_new[:, c * 48 : (c + 1) * 48],
                      in_=hst[0:SR, c * 128 : (c + 1) * 128],
                      transpose=True,
                  )

              if k == LAG - 1:
                  # splice in layer-1 initial state before iteration LAG
                  for c in range(KC):
                      nc.sync.dma_start(
                          out=xb_new[:, c * 48 + R1 : c * 48 + NR],
                          in_=ht0[:, c * 48 + R1 : c * 48 + NR],
                      )
                  nc.sync.dma_start(out=cst[R1:NR, :], in_=c0[R1:NR, :])

              if l0:
                  # collect h0^T into the wih1 window operand
                  wT = winT[(t // WIN) % 2]
                  s = t % WIN
                  for c in range(KC):
                      nc.sync.dma_start(
                          out=wT[:, c * 128 + s * 8 : c * 128 + (s + 1) * 8],
                          in_=xb_new[:, c * 48 : c * 48 + 8],
                      )
                  if s == WIN - 1:
                      # batched wih1 projection for this window (M=128)
                      wbuf = gx1buf[(t // WIN) % 2]
                      for n in range(4):
                          pw = pp.tile([128, 512], F32, tag="pw", name="pw")
                          nc.tensor.matmul(
                              out=pw[:, 0:512],
                              lhsT=ones_sb[0:1, 0:128],
                              rhs=bias1_sb[0:1, n * 512 : (n + 1) * 512],
                              start=True,
                              stop=False,
                              skip_group_check=True,
                          )
                          for c in range(KC):
                              nc.tensor.matmul(
                                  out=pw[:, 0:512],
                                  lhsT=wT[:, c * 128 : (c + 1) * 128],
                                  rhs=wih1_sb[:, c * G + n * 512 : c * G + (n + 1) * 512],
                                  start=False,
                                  stop=(c == KC - 1),
                                  skip_group_check=True,
                              )
                          nc.scalar.copy(
                              out=wbuf[:, n * 512 : (n + 1) * 512], in_=pw[:, 0:512]
                          )


_NC_CACHE = {}


def _get_nc():
    if "nc" not in _NC_CACHE:
        _NC_CACHE["nc"] = _build()
    return _NC_CACHE["nc"]


def _make_in_maps(inputs):
    tokens = np.asarray(inputs["prev_tgt_tokens"])[:, :T].astype(np.int32)  # [B, T]
    emb = np.ascontiguousarray(np.asarray(inputs["emb"], dtype=np.float32))
    W_ih = np.asarray(inputs["W_ih"], dtype=np.float32)
    W_hh = np.asarray(inputs["W_hh"], dtype=np.float32)
    b_ih = np.asarray(inputs["b_ih"], dtype=np.float32)
    b_hh = np.asarray(inputs["b_hh"], dtype=np.float32)
    hiddens = np.asarray(inputs["hiddens"], dtype=np.float32)
    cells = np.asarray(inputs["cells"], dtype=np.float32)

    def permute_gates(a, axis):
        # PyTorch order [i, f, g, o] -> kernel order [f, i, g, o]; the g
        # block is doubled so a plain sigmoid computes sigma(2 z_g).
        i, f, g, o = np.split(a, 4, axis=axis)
        return np.concatenate([f, i, 2.0 * g, o], axis=axis)

    wih0t = permute_gates(W_ih[0].T, 1).astype(np.float16)  # [D, G]
    whh0t = permute_gates(W_hh[0].T, 1).astype(np.float16)
    wih1t = permute_gates(W_ih[1].T, 1).astype(np.float16)
    whh1t = permute_gates(W_hh[1].T, 1).astype(np.float16)
    bias0 = np.ascontiguousarray(
        np.broadcast_to(
            permute_gates(b_ih[0] + b_hh[0], 0)[None, :], (128, G)
        ).astype(np.float16)
    )
    bias1 = permute_gates(b_ih[1] + b_hh[1], 0)[None, :].astype(np.float16)

    il = np.zeros((16, 64), np.float16)
    for r in range(8):
        il[r, r] = 1.0
        il[8 + r, R1 + r] = 1.0

    in_maps = []
    for core in range(N_CORES):
        sl = slice(core * BL, (core + 1) * BL)
        tok_tm = np.ascontiguousarray(tokens[sl].T.reshape(BL * T, 1))  # t-major
        ht = np.zeros((128, KC * 48), np.float16)
        for c in range(KC):
            ht[:, c * 48 : c * 48 + 8] = hiddens[0, sl, c * 128 : (c + 1) * 128].T
            ht[:, c * 48 + R1 : c * 48 + NR] = hiddens[1, sl, c * 128 : (c + 1) * 128].T
        cin = np.zeros((NR, D), np.float16)
        cin[0:BL] = cells[0, sl]
        cin[R1:NR] = cells[1, sl]
        in_maps.append(
            {
                "tokens": tok_tm,
                "emb": emb,
                "wih0t": np.ascontiguousarray(wih0t),
                "whh0t": np.ascontiguousarray(whh0t),
                "wih1t": np.ascontiguousarray(wih1t),
                "whh1t": np.ascontiguousarray(whh1t),
                "bias0": bias0,
                "bias1": np.ascontiguousarray(bias1),
                "il": il,
                "ht_init": ht,
                "c_init": cin,
            }
        )
    return in_maps


def run(inputs, trace=False, **kwargs):
    """Build (cached), run on 8 cores, return (full_output, BassKernelResults)."""
    nc = _get_nc()
    in_maps = _make_in_maps(inputs)
    res = run_bass_kernel_spmd(
        nc, in_maps, core_ids=list(range(N_CORES)), trace=trace, **kwargs
    )
    out = np.concatenate([r["out"] for r in res.results], axis=0)  # [B, T, D]
    return out.astype(np.float32), res


def kernel(**inputs) -> np.ndarray:
    out, _ = run(inputs, trace=False)
    return out



# revision 15
# speedup vs baseline: 8.4046x; 8.4046x over previous
"""Trainium2 Bass kernel: 2-layer LSTM decoder with embedding lookup.

Reference computation (per nn.Decoder):
    tgt_embed = emb[prev_tgt_tokens]                      # [B, T, D]
    for t in 0..T-1:
        x = tgt_embed[:, t]
        for l in 0..1:
            gates = x @ W_ih[l].T + b_ih[l] + h[l] @ W_hh[l].T + b_hh[l]
            i, f, g, o = split(gates, 4)
            c[l] = sigmoid(f) * c[l] + sigmoid(i) * tanh(g)
            h[l] = sigmoid(o) * tanh(c[l])
            x = h[l]
        out[:, t] = h[1]

Sharding: data-parallel over batch B=64 across 8 cores (8 rows each);
weights replicated; the sequential time loop runs on-device per core.

Kernel design v3 -- fully transposed state space:
  - All per-step tensors live as [feature, batch] (gate dim / D on the
    128-partition axis, the 8 batch rows on the free axis).  Recurrent
    matmuls then use the weight chunk as the stationary lhsT operand and
    the tiny h^T chunk as the moving rhs (N=8), so the PE streams only
    8 columns per instruction instead of 512 weight columns.
  - No per-step transposes at all: h comes out of the elementwise chain
    already transposed, and feeds the next step's matmul directly.
  - gates^T accumulate in two PSUM tiles per step (ifo | g) so the
    sigmoid can fire before the g/tanh chunks finish.
  - Gate nonlinearities use the ACT engine's Sigmoid and Tanh directly.
  - Layer 1 runs one step behind layer 0 (LAG=1); one shared
    elementwise chain processes both lanes side by side ([l, D-chunk,
    batch] views of shared tiles).
  - h history (both lanes) accumulates in a single SBUF tile, which
    doubles as the matmul operand source and the final output staging;
    the [D, batch] -> [batch, T, D] unscramble happens once at the end
    via DMA XBAR transposes.
  - Phase A (embedding gather + batched L0 input projection W_ih0 @ x^T
    + bias) runs up front at full PE tile sizes into an SBUF-resident
    gx^T buffer (no DRAM round-trip).
"""

import os

import numpy as np

import concourse.bass as bass
import concourse.mybir as mybir
import concourse.tile as tile
from concourse import bacc
from concourse.bass_utils import run_bass_kernel_spmd
from concourse.masks import make_identity

N_CORES = 8
B = 64
T = int(os.environ.get("BASS_LSTM_T", "128"))
DBG = bool(int(os.environ.get("BASS_LSTM_DBG", "0")))
D = 512
V = 32000
G = 2048            # 4*D gate dims per layer
BL = B // N_CORES   # 8 batch rows per core
KC = D // 128       # 4 contraction chunks of 128
NTOK = BL * T       # 1024 tokens per core
F32 = mybir.dt.float32
F16 = mybir.dt.float16
I32 = mybir.dt.int32
AFT = mybir.ActivationFunctionType

# gate-chunk layout (permuted host-side): chunks 0:4 = i, 4:8 = f,
# 8:12 = o, 12:16 = g.  Columns within a lane: chunk m at m*8.
NIFO = 12 * 8       # 96 ifo columns per lane
NG = 4 * 8          # 32 g columns per lane


def _build():
    nc = bacc.Bacc(
        "TRN2",
        target_bir_lowering=False,
        debug=False,
        enable_asserts=False,
        num_devices=N_CORES,
    )

    tok_d = nc.dram_tensor("tokens", [NTOK, 1], I32, kind="ExternalInput")
    emb_d = nc.dram_tensor("emb", [V, D], F16, kind="ExternalInput")
    wih0_d = nc.dram_tensor("wih0t", [D, G], F16, kind="ExternalInput")
    whh0_d = nc.dram_tensor("whh0t", [D, G], F16, kind="ExternalInput")
    wih1_d = nc.dram_tensor("wih1t", [D, G], F16, kind="ExternalInput")
    whh1_d = nc.dram_tensor("whh1t", [D, G], F16, kind="ExternalInput")
    bias0_d = nc.dram_tensor("bias0t", [128, 16], F32, kind="ExternalInput")
    bias1_d = nc.dram_tensor("bias1t", [16, 128], F16, kind="ExternalInput")
    sel_d = nc.dram_tensor("sel", [16, 128], F16, kind="ExternalInput")
    ht0_d = nc.dram_tensor("ht_init", [128, 32], F16, kind="ExternalInput")
    h1t_d = nc.dram_tensor("h1t_init", [128, 32], F16, kind="ExternalInput")
    c_d = nc.dram_tensor("c_init", [128, 64], F16, kind="ExternalInput")
    out_d = nc.dram_tensor("out", [BL, T, D], F16, kind="ExternalOutput")
    dbg_d = (
        nc.dram_tensor("dbg", [128, (T + 2) * 64 + 64 + 16 * NTOK + 640], F16, kind="ExternalOutput")
        if DBG else None
    )

    with tile.TileContext(nc) as tc:
        _body(
            tc,
            tok=tok_d.ap(),
            emb=emb_d.ap(),
            w=[wih0_d.ap(), whh0_d.ap(), wih1_d.ap(), whh1_d.ap()],
            bias0=bias0_d.ap(),
            bias1=bias1_d.ap(),
            sel=sel_d.ap(),
            ht0=ht0_d.ap(),
            h1t0=h1t_d.ap(),
            c0=c_d.ap(),
            out=out_d.ap(),
            dbg=dbg_d.ap() if DBG else None,
        )
    nc.compile()
    return nc


def _body(tc, tok, emb, w, bias0, bias1, sel, ht0, h1t0, c0, out, dbg=None):
    nc = tc.nc
    with (
        tc.tile_pool(name="wpool", bufs=1) as wp,
        tc.tile_pool(name="state", bufs=1) as st,
        tc.tile_pool(name="work", bufs=2) as wk,
        tc.tile_pool(name="pspool", bufs=1, space="PSUM") as pp,
    ):
        # ---- persistent tiles -------------------------------------------
        id16 = wp.tile([128, 128], F16)
        make_identity(nc, id16[:])

        whh0_sb = wp.tile([128, KC * G], F16)
        wih1_sb = wp.tile([128, KC * G], F16)
        whh1_sb = wp.tile([128, KC * G], F16)
        for dst, src in ((whh0_sb, w[1]), (wih1_sb, w[2]), (whh1_sb, w[3])):
            for c in range(KC):
                nc.sync.dma_start(
                    out=dst[:, c * G : (c + 1) * G],
                    in_=src.rearrange("(c p) n -> p c n", p=128)[:, c, :],
                )

        bias1_sb = wp.tile([16, 128], F16)
        nc.sync.dma_start(out=bias1_sb[:], in_=bias1)
        sel_sb = wp.tile([16, 128], F16)
        nc.sync.dma_start(out=sel_sb[:], in_=sel)
        bias0_sb = wp.tile([128, 16], F32)
        nc.sync.dma_start(out=bias0_sb[:], in_=bias0)

        # gx^T for L0 over all T steps: [128, m-chunk (16) x token (1024)]
        gxT = wp.tile([128, 16 * NTOK], F16)
        # h history, both lanes: slot k = cols (k+1)*64, lane0 h0T(k) at
        # +0:32, lane1 h1T(k-1) at +32:64.  Iter k reads slot k*64.
        hist = wp.tile([128, (T + 2) * 64], F16)
        nc.vector.memset(hist[:], 0.0)
        nc.sync.dma_start(out=hist[:, 0:32], in_=ht0)
        nc.sync.dma_start(out=hist[:, 96:128], in_=h1t0)

        # LSTM state + chain tiles ([l, chunk, batch] views).
        c_sb = st.tile([128, 64], F16)
        nc.sync.dma_start(out=c_sb[:], in_=c0)
        gt = [st.tile([128, 192], F16, name=f"gt{i}") for i in range(2)]
        gg = [st.tile([128, 64], F16, name=f"gg{i}") for i in range(2)]
        th = [st.tile([128, 64], F16, name=f"th{i}") for i in range(2)]
        fc_sb = st.tile([128, 64], F16)
        ig_sb = st.tile([128, 64], F16)

        # ---- phase A: gather + transpose + batched L0 input projection --
        with (
            tc.tile_pool(name="ph0", bufs=1) as p0,
            tc.tile_pool(name="ph0ps", bufs=1, space="PSUM") as pps0,
        ):
            wih0_sb = p0.tile([128, KC * G], F16)
            for c in range(KC):
                nc.sync.dma_start(
                    out=wih0_sb[:, c * G : (c + 1) * G],
                    in_=w[0].rearrange("(c p) n -> p c n", p=128)[:, c, :],
                )

            TT = min(512, NTOK)   # tokens per projection tile
            GT = min(128, TT)     # tokens per gather group
            for mt in range(NTOK // TT):
                embT = p0.tile([128, KC * TT], F16, tag="embT", bufs=2)
                for g4 in range(TT // GT):
                    base = mt * TT + g4 * GT
                    idx_m = p0.tile([GT, 1], I32, tag="idx", bufs=2)
                    nc.sync.dma_start(out=idx_m[:], in_=tok[base : base + GT, :])
                    emb_m = p0.tile([GT, D], F16, tag="embrows", bufs=2)
                    nc.gpsimd.indirect_dma_start(
                        out=emb_m[:],
                        out_offset=None,
                        in_=emb,
                        in_offset=bass.IndirectOffsetOnAxis(ap=idx_m[:, :1], axis=0),
                    )
                    pst = pps0.tile([128, KC * GT], F16, tag="pst", bufs=2)
                    for c in range(KC):
                        nc.tensor.transpose(
                            out=pst[:, c * GT : (c + 1) * GT],
                            in_=emb_m[:, c * 128 : (c + 1) * 128],
                            identity=id16[:GT, :GT],
                        )
                    for c in range(KC):
                        nc.vector.tensor_copy(
                            out=embT[:, c * TT + g4 * GT : c * TT + (g4 + 1) * GT],
                            in_=pst[:, c * GT : (c + 1) * GT],
                        )
                for m in range(16):  # gate chunks
                    psA = pps0.tile([128, TT], F32, tag="psA", bufs=2)
                    for c in range(KC):
                        nc.tensor.matmul(
                            out=psA[:, 0:TT],
                            lhsT=wih0_sb[:, c * G + m * 128 : c * G + (m + 1) * 128],
                            rhs=embT[:, c * TT : (c + 1) * TT],
                            start=(c == 0),
                            stop=(c == KC - 1),
                        )
                    nc.scalar.activation(
                        out=gxT[:, m * NTOK + mt * TT : m * NTOK + (mt + 1) * TT],
                        in_=psA[:, 0:TT],
                        func=AFT.Identity,
                        bias=bias0_sb[:, m : m + 1],
                    )

        # ---- phase B: recurrent loop ------------------------------------
        pifo = [pp.tile([128, 192], F32, name="pifo0")]
        pg = [pp.tile([128, 64], F32, name="pg0")]
        gxT_v = gxT[:].rearrange("p (m n) -> p m n", m=16)
        for k in range(T + 1):
            l0 = k < T
            l1 = k >= 1
            ki = k % 2
            pi, pgk = pifo[0], pg[0]
            rd = k * 64       # history slot read (h0T(k-1) | h1T(k-2))
            wr = (k + 1) * 64  # history slot written

            # -- matmuls: seeds then ifo accumulation, then g --------------
            if l0:
                nc.tensor.matmul(
                    out=pi[:, 0:NIFO],
                    lhsT=id16[:],
                    rhs=gxT_v[:, 0:12, k * 8 : (k + 1) * 8],
                    start=True, stop=False, skip_group_check=True,
                )
            if l1:
                nc.tensor.matmul(
                    out=pi[:, 96:192],
                    lhsT=bias1_sb[:],
                    rhs=sel_sb[:, 0:NIFO],
                    start=not l0, stop=False, skip_group_check=True,
                )
            for m in range(12):  # ifo chunks
                if l0:
                    for c in range(KC):
                        nc.tensor.matmul(
                            out=pi[:, m * 8 : (m + 1) * 8],
                            lhsT=whh0_sb[:, c * G + m * 128 : c * G + (m + 1) * 128],
                            rhs=hist[:, rd + c * 8 : rd + (c + 1) * 8],
                            start=False, stop=(c == KC - 1), skip_group_check=True,
                        )
                if l1:
                    for c in range(KC):
                        nc.tensor.matmul(
                            out=pi[:, 96 + m * 8 : 96 + (m + 1) * 8],
                            lhsT=wih1_sb[:, c * G + m * 128 : c * G + (m + 1) * 128],
                            rhs=hist[:, rd + c * 8 : rd + (c + 1) * 8],
                            start=False, stop=False, skip_group_check=True,
                        )
                    for c in range(KC):
                        nc.tensor.matmul(
                            out=pi[:, 96 + m * 8 : 96 + (m + 1) * 8],
                            lhsT=whh1_sb[:, c * G + m * 128 : c * G + (m + 1) * 128],
                            rhs=hist[:, rd + 32 + c * 8 : rd + 32 + (c + 1) * 8],
                            start=False, stop=(c == KC - 1), skip_group_check=True,
                        )
            if l0:
                nc.tensor.matmul(
                    out=pgk[:, 0:NG],
                    lhsT=id16[:],
                    rhs=gxT_v[:, 12:16, k * 8 : (k + 1) * 8],
                    start=True, stop=False, skip_group_check=True,
                )
            if l1:
                nc.tensor.matmul(
                    out=pgk[:, 32:64],
                    lhsT=bias1_sb[:],
                    rhs=sel_sb[:, 96:128],
                    start=not l0, stop=False, skip_group_check=True,
                )
            for m in range(12, 16):  # g chunks
                mg = m - 12
                if l0:
                    for c in range(KC):
                        nc.tensor.matmul(
                            out=pgk[:, mg * 8 : (mg + 1) * 8],
                            lhsT=whh0_sb[:, c * G + m * 128 : c * G + (m + 1) * 128],
                            rhs=hist[:, rd + c * 8 : rd + (c + 1) * 8],
                            start=False, stop=(c == KC - 1), skip_group_check=True,
                        )
                if l1:
                    for c in range(KC):
                        nc.tensor.matmul(
                            out=pgk[:, 32 + mg * 8 : 32 + (mg + 1) * 8],
                            lhsT=wih1_sb[:, c * G + m * 128 : c * G + (m + 1) * 128],
                            rhs=hist[:, rd + c * 8 : rd + (c + 1) * 8],
                            start=False, stop=False, skip_group_check=True,
                        )
                    for c in range(KC):
                        nc.tensor.matmul(
                            out=pgk[:, 32 + mg * 8 : 32 + (mg + 1) * 8],
                            lhsT=whh1_sb[:, c * G + m * 128 : c * G + (m + 1) * 128],
                            rhs=hist[:, rd + 32 + c * 8 : rd + 32 + (c + 1) * 8],
                            start=False, stop=(c == KC - 1), skip_group_check=True,
                        )

            # -- elementwise chain (lane slices la:lb) ---------------------
            la, lb = (0, 2) if (l0 and l1) else ((0, 1) if l0 else (1, 2))
            pi_v = pi[:].rearrange("p (l x) -> p l x", l=2)[:, la:lb, :]
            pg_v = pgk[:].rearrange("p (l x) -> p l x", l=2)[:, la:lb, :]
            gt_v = gt[ki][:].rearrange("p (l x) -> p l x", l=2)[:, la:lb, :]
            gg_v = gg[ki][:].rearrange("p (l x) -> p l x", l=2)[:, la:lb, :]
            th_v = th[ki][:].rearrange("p (l x) -> p l x", l=2)[:, la:lb, :]
            c_v = c_sb[:].rearrange("p (l x) -> p l x", l=2)[:, la:lb, :]
            fc_v = fc_sb[:].rearrange("p (l x) -> p l x", l=2)[:, la:lb, :]
            ig_v = ig_sb[:].rearrange("p (l x) -> p l x", l=2)[:, la:lb, :]
            ho_v = (
                hist[:, wr + la * 32 : wr + lb * 32]
                .rearrange("p (l x) -> p l x", l=lb - la)
            )

            nc.scalar.activation(out=gt_v, in_=pi_v, func=AFT.Sigmoid)
            nc.scalar.activation(out=gg_v, in_=pg_v, func=AFT.Tanh)
            nc.vector.tensor_mul(out=fc_v, in0=gt_v[:, :, 32:64], in1=c_v)
            nc.vector.tensor_mul(out=ig_v, in0=gt_v[:, :, 0:32], in1=gg_v)
            nc.vector.tensor_add(out=c_v, in0=fc_v, in1=ig_v)
            nc.scalar.activation(out=th_v, in_=c_v, func=AFT.Tanh)
            nc.vector.tensor_mul(out=ho_v, in0=gt_v[:, :, 64:96], in1=th_v)

        # ---- final: unscramble h1 history to out[b, t, d] ----------------
        hist_v = hist[:].rearrange("p (t x) -> p t x", x=64)
        for b in range(BL):
            outF = wk.tile([128, 512], F16, tag="outF", bufs=2)
            for c in range(KC):
                psf = pp.tile([128, 128], F16, tag="psf", bufs=1, name="psf")
                nc.tensor.transpose(
                    out=psf[:T, :],
                    in_=hist_v[:, 2 : T + 2, 32 + c * 8 + b],
                    identity=id16[:],
                )
                nc.vector.tensor_copy(
                    out=outF[:T, c * 128 : (c + 1) * 128], in_=psf[:T, :]
                )
            nc.sync.dma_start(out=out[b], in_=outF[:T, :])
        if dbg is not None:
            nc.sync.dma_start(out=dbg[:, : (T + 2) * 64], in_=hist[:])
            nc.sync.dma_start(out=dbg[:, (T + 2) * 64 : (T + 2) * 64 + 64], in_=c_sb[:])
            nc.sync.dma_start(
                out=dbg[:, (T + 2) * 64 + 64 : (T + 2) * 64 + 64 + 16 * NTOK],
                in_=gxT[:],
            )
            dx = (T + 2) * 64 + 64 + 16 * NTOK
            nc.sync.dma_start(out=dbg[:, dx : dx + 192], in_=gt[1][:])
            nc.sync.dma_start(out=dbg[:, dx + 192 : dx + 256], in_=gg[1][:])
            nc.sync.dma_start(out=dbg[:, dx + 256 : dx + 320], in_=th[1][:])


_NC_CACHE = {}


def _get_nc():
    if "nc" not in _NC_CACHE:
        _NC_CACHE["nc"] = _build()
    return _NC_CACHE["nc"]


# PyTorch gate blocks [i, f, g, o] -> kernel order [i, f, o, g]
_PERM = np.r_[0:512, 512:1024, 1536:2048, 1024:1536]


def _tr8(x):
    """[8, 512] -> [128, 32] transposed chunk layout (cols = c*8 + b)."""
    return np.ascontiguousarray(
        x.reshape(BL, KC, 128).transpose(2, 1, 0).reshape(128, KC * BL)
    ).astype(np.float16)


def _make_in_maps(inputs):
    tokens = np.asarray(inputs["prev_tgt_tokens"])[:, :T].astype(np.int32)  # [B, T]
    emb = np.ascontiguousarray(np.asarray(inputs["emb"], dtype=np.float16))
    W_ih = np.asarray(inputs["W_ih"], dtype=np.float32)
    W_hh = np.asarray(inputs["W_hh"], dtype=np.float32)
    b_ih = np.asarray(inputs["b_ih"], dtype=np.float32)
    b_hh = np.asarray(inputs["b_hh"], dtype=np.float32)
    hiddens = np.asarray(inputs["hiddens"], dtype=np.float32)
    cells = np.asarray(inputs["cells"], dtype=np.float32)

    wts = {}
    for name, wmat in (
        ("wih0t", W_ih[0]), ("whh0t", W_hh[0]),
        ("wih1t", W_ih[1]), ("whh1t", W_hh[1]),
    ):
        wts[name] = np.ascontiguousarray(wmat.T[:, _PERM]).astype(np.float16)
    bias_l0 = (b_ih[0] + b_hh[0])[_PERM]
    bias_l1 = (b_ih[1] + b_hh[1])[_PERM]
    bias0t = np.ascontiguousarray(bias_l0.reshape(16, 128).T).astype(np.float32)
    bias1t = np.ascontiguousarray(bias_l1.reshape(16, 128)).astype(np.float16)
    sel = np.kron(np.eye(16), np.ones((1, 8))).astype(np.float16)

    in_maps = []
    for core in range(N_CORES):
        sl = slice(core * BL, (core + 1) * BL)
        tok_tm = np.ascontiguousarray(tokens[sl].T.reshape(NTOK, 1))  # t-major
        cin = np.concatenate([_tr8(cells[0, sl]), _tr8(cells[1, sl])], axis=1)
        in_maps.append(
            {
                "tokens": tok_tm,
                "emb": emb,
                **wts,
                "bias0t": bias0t,
                "bias1t": bias1t,
                "sel": sel,
                "ht_init": _tr8(hiddens[0, sl]),
                "h1t_init": _tr8(hiddens[1, sl]),
                "c_init": np.ascontiguousarray(cin),
            }
        )
    return in_maps


def run(inputs, trace=False, **kwargs):
    """Build (cached), run on 8 cores, return (full_output, BassKernelResults)."""
    nc = _get_nc()
    in_maps = _make_in_maps(inputs)
    res = run_bass_kernel_spmd(
        nc, in_maps, core_ids=list(range(N_CORES)), trace=trace, **kwargs
    )
    out = np.concatenate([r["out"] for r in res.results], axis=0)  # [B, T, D]
    return out.astype(np.float32), res


def kernel(**inputs) -> np.ndarray:
    out, _ = run(inputs, trace=False)
    return out
